# revision 17
# baseline (speedup 1.0000x reference)
"""AttentionWithContextV2 pooling kernel for 8 Trainium2 NeuronCores.

Math (per batch b):
    uit = tanh(x @ W + b)            [T, F]
    ait = uit @ u.T                  [T, C]
    e   = exp(ait) * mask[:, None]   [T, C]
    a   = e / (sum_t e + EPS)        [T, C]
    out = a.T @ x                    [C, F]
    scores = a.T                     [C, T]

Sharding: data-parallel over batch B=64 across 8 cores (8 batches/core).
W/b/u replicated. No cross-device comms.

Device strategy (per core, batches in pairs packing C=64 onto 128
partitions):
  - Host pre-packs x in two bf16 layouts: xt (F on partitions, for x@W)
    and xn (T on partitions, for a.T@x); together they equal one fp32
    read of x, and the device does zero transposes of x.
  - mm1: uitT = (xW)^T per 512-wide T chunk on PE, tanh+bias on ACT.
  - mm2: aitT[c,t] packed pair-wise on partitions; exp on ACT with
    accum_out producing the softmax denominator.
  - After the pair's denominator is known, eT is scaled once (DVE), PE
    transposes a^T back to natural layout, and a.T@x accumulates on PE.
  - Emission is stage-major with a one-chunk skew for exp and a
    one-pair skew for the normalize/transpose/a.T@x phase, so each
    engine's in-order instruction stream always has ready work.

Mask handling: when mask is not all-ones the host sends an additive
bias row Bm[t] in {0, -10000}; a rank-1 (K=1) matmul adds it to ait
before exp, making masked e exactly 0 (exp underflow).
"""

import numpy as np
import ml_dtypes

BF16_NP = ml_dtypes.bfloat16

B, T, F, C = 64, 2048, 256, 64
NCORES = 8
BSH = B // NCORES            # batches per core
NPAIR = BSH // 2             # batch pairs per core
TCH = 512                    # T chunk size
NCH = T // TCH               # chunks per batch
KF = F // 128                # K chunks for the F contraction
EPS = 1e-7

_prog_cache = {}


def _build(apply_mask: bool, reps: int = 1, internal: bool = False, mode: str = "full",
           big_dma: bool = False, et_bf16: bool = False):
    do_dma = mode in ("full", "dma")
    do_compute = mode in ("full", "compute")
    import concourse.bacc as bacc
    import concourse.tile as tile
    import concourse.mybir as mybir
    from concourse.masks import make_identity

    fp32 = mybir.dt.float32
    bf16 = mybir.dt.bfloat16
    AF = mybir.ActivationFunctionType

    nc = bacc.Bacc("TRN2", target_bir_lowering=False, debug=False)

    ikind = "Internal" if internal else "ExternalInput"
    okind = "Internal" if internal else "ExternalOutput"
    xt_h = nc.dram_tensor("xt", [BSH, NCH, 128, KF, TCH], bf16, kind=ikind)
    xn_h = nc.dram_tensor("xn", [BSH, NCH, 128, TCH // 128, F], bf16, kind=ikind)
    w_h = nc.dram_tensor("wp", [128, KF, F], bf16, kind=ikind)
    ut_h = nc.dram_tensor("utp", [128, KF, C], bf16, kind=ikind)
    b_h = nc.dram_tensor("bp", [128, KF], fp32, kind=ikind)
    if apply_mask:
        bm_h = nc.dram_tensor("bm", [BSH, NCH, TCH], bf16, kind=ikind)
    out_h = nc.dram_tensor("out", [BSH, C, F], fp32, kind=okind)
    et_dt = bf16 if et_bf16 else fp32
    sc_h = nc.dram_tensor("scores", [BSH, C, T], et_dt, kind=okind)
    if internal:
        okv_h = nc.dram_tensor("okv", [1, 1], fp32, kind="ExternalOutput")

    with tile.TileContext(nc) as tc:
        with (
            tc.tile_pool(name="consts", bufs=1) as consts,
            tc.tile_pool(name="xin", bufs=4 if big_dma else 6) as xin,
            tc.tile_pool(name="xnp", bufs=5 if big_dma else 18) as xnp,
            tc.tile_pool(name="uip", bufs=11) as uip,
            tc.tile_pool(name="enp", bufs=4) as enp,
            tc.tile_pool(name="etp", bufs=3) as etp,
            tc.tile_pool(name="small", bufs=12) as small,
            tc.tile_pool(name="outp", bufs=2) as outp,
            tc.tile_pool(name="ps_mm", bufs=2, space="PSUM") as ps_mm,
            tc.tile_pool(name="ps_at", bufs=2, space="PSUM") as ps_at,
            tc.tile_pool(name="ps_en", bufs=2, space="PSUM") as ps_en,
            tc.tile_pool(name="ps_ou", bufs=2, space="PSUM") as ps_ou,
        ):
            w_sb = consts.tile([128, KF, F], bf16)
            nc.sync.dma_start(out=w_sb, in_=w_h[:])
            ut_sb = consts.tile([128, KF, C], bf16)
            nc.sync.dma_start(out=ut_sb, in_=ut_h[:])
            b_sb = consts.tile([128, KF], fp32)
            nc.sync.dma_start(out=b_sb, in_=b_h[:])
            id_sb = consts.tile([128, 128], et_dt)
            make_identity(nc, id_sb)
            if apply_mask:
                ones_sb = consts.tile([1, C], bf16)
                nc.vector.memset(ones_sb, 1.0)

            def load_batch(st, bi):
                """big_dma: one whole-batch DMA per layout."""
                bb = st["b0"] + bi
                xt_sb = xin.tile([128, NCH, KF, TCH], bf16, tag="xt", name="xtb")
                if do_dma:
                    nc.sync.dma_start(
                        out=xt_sb, in_=xt_h[bb].rearrange("c p k t -> p c k t")
                    )
                else:
                    nc.gpsimd.memset(xt_sb[:, 0, 0, 0:1], 0.5)
                xn_sb = xnp.tile([128, NCH, TCH // 128, F], bf16, tag="xn", name="xnb")
                if do_dma:
                    nc.sync.dma_start(
                        out=xn_sb, in_=xn_h[bb].rearrange("c p j f -> p c j f")
                    )
                elif do_compute:
                    nc.gpsimd.memset(xn_sb[:, 0, 0, 0:1], 0.5)
                st["xtb"][bi] = xt_sb
                st["xnb"][bi] = xn_sb

            def stage1_chunk(st, c):
                """Load x, mm1, tanh for both batches of chunk c."""
                for bi in range(2):
                    bb = st["b0"] + bi
                    if big_dma:
                        xt_sb = st["xtb"][bi][:, c]
                        st["xn"][(c, bi)] = st["xnb"][bi][:, c]
                    else:
                        xt_sb = xin.tile([128, KF, TCH], bf16, tag="xt")
                        if do_dma:
                            nc.sync.dma_start(out=xt_sb, in_=xt_h[bb, c])
                        else:
                            nc.gpsimd.memset(xt_sb[:, 0, 0:1], 0.5)
                        xn_sb = xnp.tile([128, TCH // 128, F], bf16, tag="xn")
                        if do_dma:
                            nc.sync.dma_start(out=xn_sb, in_=xn_h[bb, c])
                        elif do_compute:
                            nc.gpsimd.memset(xn_sb[:, 0, 0:1], 0.5)
                        st["xn"][(c, bi)] = xn_sb
                    if not do_compute:
                        continue
                    ui_sb = uip.tile([128, KF, TCH], bf16, tag="ui")
                    st["ui"][(c, bi)] = ui_sb
                    for g in range(2):
                        mm_ps = ps_mm.tile([128, TCH], fp32)
                        for k in range(KF):
                            nc.tensor.matmul(
                                mm_ps,
                                lhsT=w_sb[:, k, g * 128:(g + 1) * 128],
                                rhs=xt_sb[:, k, :],
                                start=(k == 0),
                                stop=(k == KF - 1),
                            )
                        nc.scalar.activation(
                            ui_sb[:, g, :], mm_ps, AF.Tanh,
                            bias=b_sb[:, g:g + 1],
                        )

            def stage2_chunk(st, c):
                """mm2 (+mask bias) and exp for chunk c (batch pair packed)."""
                if not do_compute:
                    return
                at_ps = ps_at.tile([128, TCH], fp32)
                for bi in range(2):
                    ui_sb = st["ui"].pop((c, bi))
                    for k in range(KF):
                        nc.tensor.matmul(
                            at_ps[bi * 64:(bi + 1) * 64, :],
                            lhsT=ut_sb[:, k, :],
                            rhs=ui_sb[:, k, :],
                            start=(k == 0),
                            stop=(k == KF - 1 and not apply_mask),
                            skip_group_check=True,
                        )
                    if apply_mask:
                        bm_sb = small.tile([1, TCH], bf16, tag="bm")
                        nc.sync.dma_start(out=bm_sb, in_=bm_h[st["b0"] + bi, c])
                        nc.tensor.matmul(
                            at_ps[bi * 64:(bi + 1) * 64, :],
                            lhsT=ones_sb,
                            rhs=bm_sb,
                            start=False,
                            stop=True,
                            skip_group_check=True,
                        )
                nc.scalar.activation(
                    st["eT"][:, c * TCH:(c + 1) * TCH], at_ps, AF.Exp,
                    accum_out=st["acc"][:, c:c + 1],
                )

            def stage12(pr):
                st = {
                    "b0": 2 * pr,
                    "eT": etp.tile([128, T], et_dt, tag="eT", name="eT"),
                    "acc": small.tile([128, NCH], fp32, tag="acc", name="acc"),
                    "xn": {},
                    "ui": {},
                    "xtb": {},
                    "xnb": {},
                }
                if big_dma:
                    load_batch(st, 0)
                    load_batch(st, 1)
                for c in range(NCH):
                    stage1_chunk(st, c)
                    if c >= 1:
                        stage2_chunk(st, c - 1)
                stage2_chunk(st, NCH - 1)
                return st

            last_out_sb = None

            def stage345(st):
                nonlocal last_out_sb
                b0 = st["b0"]
                eT = st["eT"]
                out_sb = outp.tile([128, F], fp32, tag="osb")
                if do_compute:
                    s_sum = small.tile([128, 1], fp32, tag="ssum")
                    nc.vector.reduce_sum(
                        s_sum, st["acc"], axis=mybir.AxisListType.X
                    )
                    nc.vector.tensor_scalar_add(s_sum, s_sum, EPS)
                    recip = small.tile([128, 1], fp32, tag="recip")
                    nc.vector.reciprocal(recip, s_sum)
                    nc.vector.tensor_scalar_mul(eT, eT, recip)
                    ou_ps = ps_ou.tile([128, F], fp32)
                    for c in range(NCH):
                        for bi in range(2):
                            en_ps = ps_en.tile([128, 4 * C], et_dt)
                            for j in range(TCH // 128):
                                nc.tensor.transpose(
                                    en_ps[:, j * C:(j + 1) * C],
                                    eT[bi * 64:(bi + 1) * 64,
                                       c * TCH + j * 128:c * TCH + (j + 1) * 128],
                                    id_sb[bi * 64:(bi + 1) * 64,
                                          bi * 64:(bi + 1) * 64],
                                )
                            en_sb = enp.tile([128, 4 * C], bf16, tag="en")
                            nc.vector.tensor_copy(en_sb, en_ps)
                            xn_sb = st["xn"].pop((c, bi))
                            for j in range(TCH // 128):
                                nc.tensor.matmul(
                                    ou_ps[bi * 64:(bi + 1) * 64, :],
                                    lhsT=en_sb[:, j * C:(j + 1) * C],
                                    rhs=xn_sb[:, j, :],
                                    start=(c == 0 and j == 0),
                                    stop=(c == NCH - 1 and j == TCH // 128 - 1),
                                    skip_group_check=True,
                                )
                    nc.vector.tensor_copy(out_sb, ou_ps)
                else:
                    nc.gpsimd.memset(eT[:, 0:1], 0.5)
                    nc.gpsimd.memset(out_sb[:, 0:1], 0.5)
                if do_dma:
                    nc.sync.dma_start(
                        out=sc_h[b0:b0 + 2].flatten_outer_dims(), in_=eT
                    )
                    nc.sync.dma_start(
                        out=out_h[b0:b0 + 2].flatten_outer_dims(), in_=out_sb
                    )
                last_out_sb = out_sb

            pending = None
            for idx in range(NPAIR * reps):
                st = stage12(idx % NPAIR)
                if pending is not None:
                    stage345(pending)
                pending = st
            stage345(pending)
            if internal:
                nc.sync.dma_start(out=okv_h[:], in_=last_out_sb[0:1, 0:1])

    nc.finalize()
    return nc


def get_program(apply_mask: bool):
    key = bool(apply_mask)
    if key not in _prog_cache:
        _prog_cache[key] = _build(key)
    return _prog_cache[key]


def prepare_inputs(x, W, b, u, mask):
    """Host-side packing into the per-core layouts the device expects."""
    x = np.asarray(x, dtype=np.float32)
    W = np.asarray(W, dtype=np.float32)
    b = np.asarray(b, dtype=np.float32)
    u = np.asarray(u, dtype=np.float32)
    mask = np.asarray(mask)

    apply_mask = not bool(mask.all())

    # xt[b, c, p, k, t'] = x[b, 512c + t', 128k + p]
    xt = np.ascontiguousarray(
        x.reshape(B, NCH, TCH, KF, 128).transpose(0, 1, 4, 3, 2)
    ).astype(BF16_NP)
    # xn[b, c, p, j, f] = x[b, 512c + 128j + p, f]
    xn = np.ascontiguousarray(
        x.reshape(B, NCH, TCH // 128, 128, F).transpose(0, 1, 3, 2, 4)
    ).astype(BF16_NP)
    # wp[p, k, g] = W[128k + p, g]
    wp = np.ascontiguousarray(
        W.reshape(KF, 128, F).transpose(1, 0, 2)
    ).astype(BF16_NP)
    # utp[p, k, c] = u[c, 128k + p]
    utp = np.ascontiguousarray(
        u.T.reshape(KF, 128, C).transpose(1, 0, 2)
    ).astype(BF16_NP)
    # bp[p, k] = b[128k + p]
    bp = np.ascontiguousarray(b.reshape(KF, 128).T).astype(np.float32)

    in_maps = []
    for i in range(NCORES):
        m = {
            "xt": xt[i * BSH:(i + 1) * BSH],
            "xn": xn[i * BSH:(i + 1) * BSH],
            "wp": wp,
            "utp": utp,
            "bp": bp,
        }
        if apply_mask:
            bm = np.where(mask, 0.0, -10000.0).astype(BF16_NP)
            m["bm"] = np.ascontiguousarray(
                bm[i * BSH:(i + 1) * BSH].reshape(BSH, NCH, TCH)
            )
        in_maps.append(m)
    return apply_mask, in_maps


def run(inputs, trace=False):
    """Run on the 8 NeuronCores; returns (output, attention_scores, results)."""
    from concourse.bass_utils import run_bass_kernel_spmd

    apply_mask, in_maps = prepare_inputs(**inputs)
    nc = get_program(apply_mask)
    res = run_bass_kernel_spmd(
        nc, in_maps, list(range(NCORES)), trace=trace
    )
    output = np.concatenate([r["out"] for r in res.results], axis=0)
    scores = np.concatenate(
        [np.asarray(r["scores"], dtype=np.float32) for r in res.results], axis=0
    )
    return output, scores, res


def kernel(x, W, b, u, mask):
    output, scores, _ = run(dict(x=x, W=W, b=b, u=u, mask=mask))
    return output, scores
